# revision 1
# baseline (speedup 1.0000x reference)
"""Trainium2 Bass kernel: single-head causal attention with softmax over the
QUERY axis (dim=1), as in the reference nn.Module.

Math per batch b:
    q = x Wq + bq ; k = x Wk + bk ; v = x Wv + bv          [T, H]
    S[i, j] = (q_i . k_j) * H**-0.5, masked to i >= j (tril)
    softmax over axis i (per key-column j):  s_j = sum_{i>=j} exp(S[i,j])
    out[i] = sum_j (exp(S[i,j]) / s_j) * v[j]
(no max-subtraction needed: |S| <= ~4 for this input distribution, exp is
exactly shift-invariant mathematically and fp32-safe here)

Sharding (8 cores): (batch b in 0..3) x (column-parity p in 0..1).
Core (b, p) owns key-columns j in 128-blocks {2t+p}. Column stats (s_j) are
core-local; partial outputs out^T[h, i] are summed on the host.

SPMD trick: all 8 cores run ONE program. Parity enters only through data:
 - for p=1 the host swaps adjacent 128-column blocks of x^T, so the core's
   kv-blocks always sit at even block positions (2s), and
 - a per-core [128, 512] mask handles the triangular diagonal block plus the
   (invalid for p=1) neighbour block on each strip's first chunk.
The host un-swaps the p=1 output columns before combining.
"""

import os
import sys

import numpy as np

for _p in ("/opt/trn_rl_repo", os.path.expanduser("~/.axon_site/_ro/trn_rl_repo")):
    if os.path.isdir(_p) and _p not in sys.path:
        sys.path.append(_p)

import concourse.bass as bass  # noqa: E402
import concourse.mybir as mybir  # noqa: E402
import concourse.tile as tile  # noqa: E402
from concourse import bacc  # noqa: E402

B, T_FULL, E, H = 4, 4096, 1024, 64
NCORES = 8

F32 = mybir.dt.float32
F32R = mybir.dt.float32r
BF16 = mybir.dt.bfloat16
AF = mybir.ActivationFunctionType
AX = mybir.AxisListType


def build_nc(T=T_FULL, xt_dt=F32R, seq_dt=F32R):
    """Emit the single-core SPMD program. T must be a multiple of 1024."""
    assert T % 1024 == 0
    NEC = E // 128   # contraction chunks for the projections
    NCH = T // 512   # 512-wide i-chunks
    NST = T // 256   # kv strips owned by this core (one 128-block each)
    NIP = T // 1024  # 1024-wide DMA groups (2 i-chunks each)

    nc = bacc.Bacc(
        "TRN2", target_bir_lowering=False, debug=False, num_devices=NCORES
    )
    xt = nc.dram_tensor("xt", [E, T], xt_dt, kind="ExternalInput").ap()
    # [Wq*scale | Wk | Wv] prearranged on host into SBUF layout [p, ec, col]
    wgt = nc.dram_tensor(
        "wgt", [128, NEC * 192], xt_dt, kind="ExternalInput"
    ).ap()
    # packed seq-dtype consts: mask [128, 0:512], identity [0:64, 512:576]
    cseq = nc.dram_tensor("cseq", [128, 576], seq_dt, kind="ExternalInput").ap()
    # packed f32 consts: bqk [128, 0:1], bv [0:64, 1:2]
    cf32 = nc.dram_tensor("cf32", [128, 2], F32, kind="ExternalInput").ap()
    outp = nc.dram_tensor("outp", [H, T], F32, kind="ExternalOutput").ap()
    scr = nc.dram_tensor("scr", [128, 128], F32).ap()  # warmup sink (Internal)

    def strip_len(s):
        return T - 256 * s

    def head_w(s):
        return 512 if s % 2 == 0 else 256

    def units_of(s):
        """Scoring units: head (masked), then 1024-wide exp superchunks."""
        L = strip_len(s)
        offs = [0]
        o = head_w(s)
        while o < L:
            offs.append(o)
            o += 1024
        out = []
        for i, off in enumerate(offs):
            w = min(1024 if i else head_w(s), L - off)
            ready = (256 * s + off + w - 1) // 512
            out.append((i, off, w, ready))
        return out

    with tile.TileContext(nc) as tc:
        with (
            tc.tile_pool(name="consts", bufs=1) as consts,
            tc.tile_pool(name="big", bufs=1) as big,
            tc.tile_pool(name="xtp", bufs=2) as xtp,
            tc.tile_pool(name="outb", bufs=3) as outb,
        ):
            # --- constants (3 DMAs) ---
            wgt_sb = consts.tile([128, NEC, 192], xt_dt)
            nc.sync.dma_start(
                out=wgt_sb, in_=wgt.rearrange("p (n c) -> p n c", n=NEC)
            )
            wqk_sb = wgt_sb[:, :, 0:128]
            wv_sb = wgt_sb[:, :, 128:192]
            cf32_sb = consts.tile([128, 2], F32)
            nc.sync.dma_start(out=cf32_sb, in_=cf32)
            bqk_sb = cf32_sb[:, 0:1]
            bv_sb = cf32_sb[0:64, 1:2]
            cseq_sb = consts.tile([128, 576], seq_dt)
            nc.scalar.dma_start(out=cseq_sb, in_=cseq)
            mask_sb = cseq_sb[:, 0:512]
            id_sb = cseq_sb[0:64, 512:576]

            # --- persistent intermediates ---
            qkT = big.tile([128, T], seq_dt)      # rows 0:64 q'^T, 64:128 k^T
            kT_sb = big.tile([H, NST, 128], seq_dt)  # k^T kv blocks, base-0
            vT_kv = big.tile([H, NST, 128], seq_dt)  # v^T, kv blocks only
            v_sb = big.tile([128, NST, H], seq_dt)  # v[j, h], later scaled 1/s
            e_t = [
                big.tile([128, strip_len(s)], seq_dt, tag=f"e{s}", name=f"e{s}")
                for s in range(NST)
            ]
            sums = big.tile([128, NST, 8], F32)   # per-unit exp-sum partials
            stot = big.tile([128, NST, 1], F32)
            rec = big.tile([128, NST, 1], F32)
            nc.gpsimd.memset(sums, 0.0)

            with (
                tc.tile_pool(name="pqk", bufs=2, space="PSUM") as pqk,
                tc.tile_pool(name="pvt", bufs=1, space="PSUM") as pvt,
                tc.tile_pool(name="ptr", bufs=1, space="PSUM") as ptr,
                tc.tile_pool(name="psc", bufs=2, space="PSUM") as pscp,
            ):

                def emit_unit(s, idx, off, w):
                    """Scores+exp for cols [off, off+w) of strip s."""
                    sc = pscp.tile([128, 1024], F32, tag="sc", name=f"sc{s}_{idx}")
                    for seg in range(0, w, 512):
                        sw = min(512, w - seg)
                        g = 256 * s + off + seg
                        nc.tensor.matmul(
                            sc[:, seg : seg + sw],
                            lhsT=kT_sb[:, s, :],
                            rhs=qkT[0:64, g : g + sw],
                            start=True,
                            stop=True,
                        )
                    if idx == 0:
                        nc.scalar.activation(
                            out=e_t[s][:, 0:w], in_=sc[:, 0:w], func=AF.Exp
                        )
                        nc.vector.tensor_mul(
                            e_t[s][:, 0:w], e_t[s][:, 0:w], mask_sb[:, 0:w]
                        )
                        nc.vector.reduce_sum(
                            out=sums[:, s, 0:1], in_=e_t[s][:, 0:w], axis=AX.X
                        )
                    elif 256 * s + off + w >= T - 512:
                        # dependency-critical tail: keep ACT lean, sum on DVE
                        nc.scalar.activation(
                            out=e_t[s][:, off : off + w],
                            in_=sc[:, 0:w],
                            func=AF.Exp,
                        )
                        nc.vector.reduce_sum(
                            out=sums[:, s, idx : idx + 1],
                            in_=e_t[s][:, off : off + w],
                            axis=AX.X,
                        )
                    else:
                        nc.scalar.activation(
                            out=e_t[s][:, off : off + w],
                            in_=sc[:, 0:w],
                            func=AF.Exp,
                            accum_out=sums[:, s, idx : idx + 1],
                        )

                wsink = big.tile([128, 128], F32)

                def emit_warmup(n):
                    """Dummy back-to-back matmuls to trip the PE HAM clock
                    gate to 8/8 while PE would otherwise idle."""
                    wp = ptr.tile([128, 128], F32, tag="tr", name="warm")
                    for i in range(n):
                        nc.tensor.matmul(
                            wp,
                            lhsT=wgt_sb[:, 0, 0:128],
                            rhs=wgt_sb[:, 0, 0:128],
                            start=(i == 0),
                            stop=(i == n - 1),
                        )
                    nc.vector.tensor_copy(out=wsink, in_=wp)

                emit_warmup(20)

                units_by_ic = {}
                for s in range(NST):
                    for (i, off, w, ready) in units_of(s):
                        units_by_ic.setdefault(ready, []).append((s, i, off, w))

                def emit_units_of_ic(ic):
                    for (s_, i, off, w) in units_by_ic.get(ic, []):
                        emit_unit(s_, i, off, w)

                # --- phase 1, with scoring delayed one group for slack ---
                for icp in range(NIP):
                    xts = xtp.tile(
                        [128, NEC, 1024], xt_dt, tag="xt", name=f"xts{icp}"
                    )
                    xt_r = xt.rearrange("(n p) t -> p n t", p=128)[
                        :, :, 1024 * icp : 1024 * (icp + 1)
                    ]
                    if icp == 0:
                        # split first load so matmuls start sooner
                        for ec in range(NEC):
                            nc.sync.dma_start(
                                out=xts[:, ec, :], in_=xt_r[:, ec, :]
                            )

                    else:
                        nc.sync.dma_start(out=xts, in_=xt_r)
                    for sub in range(2):
                        ic = 2 * icp + sub
                        qk_ps = pqk.tile([128, 512], F32, tag="qk")
                        vt_ps = pvt.tile([H, 256], F32, tag="vt")
                        for ec in range(NEC):
                            rhs = xts[:, ec, 512 * sub : 512 * sub + 512]
                            nc.tensor.matmul(
                                qk_ps,
                                lhsT=wqk_sb[:, ec, :],
                                rhs=rhs,
                                start=(ec == 0),
                                stop=(ec == NEC - 1),
                            )
                            kv_rhs = rhs.rearrange("p (b x) -> p b x", b=2)[
                                :, :, 0:128
                            ]
                            nc.tensor.matmul(
                                vt_ps,
                                lhsT=wv_sb[:, ec, :],
                                rhs=kv_rhs,
                                start=(ec == 0),
                                stop=(ec == NEC - 1),
                            )
                        nc.vector.tensor_scalar_add(
                            out=qkT[:, 512 * ic : 512 * (ic + 1)],
                            in0=qk_ps,
                            scalar1=bqk_sb,
                        )
                        nc.vector.tensor_scalar_add(
                            out=vT_kv[:, 2 * ic : 2 * ic + 2, :],
                            in0=vt_ps,
                            scalar1=bv_sb,
                        )
                        # k^T blocks of this chunk to base-0 (one DMA, 2 strips)
                        nc.sync.dma_start(
                            out=kT_sb[:, 2 * ic : 2 * ic + 2, :],
                            in_=qkT[
                                64:128, 512 * ic : 512 * (ic + 1)
                            ].rearrange("p (b x) -> p b x", b=2)[:, :, 0:128],
                        )
                        emit_units_of_ic(ic)

                # --- v transposes (PE idles while last exps run) ---
                for g in range(NST // 4):
                    tr = ptr.tile([128, 4, H], seq_dt, tag="tr")
                    for m in range(4):
                        nc.tensor.transpose(
                            tr[:, m, :], vT_kv[:, 4 * g + m, :], id_sb
                        )
                    nc.vector.tensor_copy(
                        out=v_sb[:, 4 * g : 4 * g + 4, :], in_=tr
                    )

                emit_warmup(16)
                # --- finalize column stats, scale v ---
                nc.vector.reduce_sum(out=stot, in_=sums, axis=AX.X)
                nc.vector.reciprocal(out=rec, in_=stot)
                for s in range(NST):
                    nc.vector.tensor_scalar_mul(
                        out=v_sb[:, s, :], in0=v_sb[:, s, :], scalar1=rec[:, s, :]
                    )

            # --- output: out^T[h, i] = sum_s v'[s]^T e[s], two chunks
            # computed concurrently in PE column groups h0/h1 ---
            with tc.tile_pool(name="pout", bufs=2, space="PSUM") as pout:
                for cp in range(NCH // 2):
                    c0, c1 = 2 * cp, 2 * cp + 1
                    op = pout.tile([128, 512], F32, tag="op")
                    mms = []
                    for s in range(2 * c1 + 2):
                        for half, c in ((0, c0), (1, c1)):
                            if s > 2 * c + 1:
                                continue
                            soff = 512 * c - 256 * s
                            rows = op[64 * half : 64 * half + 64, :]
                            if soff >= 0:
                                mms.append(
                                    (half, rows, s, e_t[s][:, soff : soff + 512])
                                )
                            else:
                                mms.append(
                                    (half, rows[:, 256:512], s, e_t[s][:, 0:256])
                                )
                    seen_half = set()
                    for i, (half, dst, s, rhs) in enumerate(mms):
                        first = half not in seen_half
                        seen_half.add(half)
                        nc.tensor.matmul(
                            dst,
                            lhsT=v_sb[:, s, :],
                            rhs=rhs,
                            start=first,
                            stop=(i == len(mms) - 1),
                            skip_group_check=True,
                        )
                    ob = outb.tile([128, 512], F32, tag="ob")
                    nc.scalar.activation(out=ob, in_=op, func=AF.Copy)
                    for m in range(2):
                        nc.sync.dma_start(
                            out=outp[
                                :, 512 * (2 * cp + m) : 512 * (2 * cp + m + 1)
                            ],
                            in_=ob[64 * m : 64 * m + 64, :],
                        )

            nc.sync.dma_start(out=scr, in_=wsink)

    nc.compile()
    return nc


def _make_mask(parity):
    m = np.zeros((128, 512), np.float32)
    m[:, 0:128] = np.tri(128, dtype=np.float32).T  # valid: i_off >= j_off
    if parity == 0:
        m[:, 128:256] = 1.0
    m[:, 256:512] = 1.0
    return m


def _swap_blocks_cols(a, blk=128):
    """Swap adjacent blk-wide column blocks: [..., 2t | 2t+1] -> [2t+1 | 2t]."""
    n = a.shape[-1]
    v = a.reshape(*a.shape[:-1], n // (2 * blk), 2, blk)
    return np.ascontiguousarray(v[..., ::-1, :].reshape(a.shape))


def host_prepare(x, Wq, bq, Wk, bk, Wv, bv, T=T_FULL, xt_dt=None, seq_dt=None):
    xt_np = mybir.dt.np(xt_dt if xt_dt is not None else XT_DT)
    seq_np = mybir.dt.np(seq_dt if seq_dt is not None else SEQ_DT)
    scale = np.float32(H**-0.5)
    # [Wq*scale | Wk | Wv] -> SBUF layout [p, ec*192 + col]
    wcat = np.concatenate(
        [
            np.asarray(Wq, np.float32) * scale,
            np.asarray(Wk, np.float32),
            np.asarray(Wv, np.float32),
        ],
        axis=1,
    )  # [E, 192]
    wgt_h = np.ascontiguousarray(
        wcat.reshape(8, 128, 192).transpose(1, 0, 2).reshape(128, 8 * 192)
    ).astype(xt_np)
    cf32_h = np.zeros((128, 2), np.float32)
    cf32_h[:, 0] = np.concatenate(
        [np.asarray(bq, np.float32) * scale, np.asarray(bk, np.float32)]
    )
    cf32_h[0:H, 1] = np.asarray(bv, np.float32)
    cseq = np.zeros((128, 576), np.float32)
    cseq[0:H, 512:576] = np.eye(H, dtype=np.float32)
    cseq_m = [cseq.copy(), cseq.copy()]
    for p in (0, 1):
        cseq_m[p][:, 0:512] = _make_mask(p)
    in_maps = []
    for core in range(NCORES):
        b, p = divmod(core, 2)
        xt_h = np.ascontiguousarray(np.asarray(x[b], np.float32).T)  # [E, T]
        if p == 1:
            xt_h = _swap_blocks_cols(xt_h)
        in_maps.append(
            {
                "xt": xt_h.astype(xt_np),
                "wgt": wgt_h,
                "cseq": cseq_m[p].astype(seq_np),
                "cf32": cf32_h,
            }
        )
    return in_maps


def host_combine(results, T=T_FULL):
    out = np.zeros((B, T, H), np.float32)
    for b in range(B):
        o0 = np.asarray(results[2 * b]["outp"])
        o1 = _swap_blocks_cols(np.asarray(results[2 * b + 1]["outp"]))
        out[b] = (o0 + o1).T
    return out


_NC_CACHE = {}

# active dtype mode for matmul operands (PSUM accumulation stays fp32)
XT_DT = BF16
SEQ_DT = BF16


def get_nc(T=T_FULL, xt_dt=None, seq_dt=None):
    key = (
        T,
        xt_dt if xt_dt is not None else XT_DT,
        seq_dt if seq_dt is not None else SEQ_DT,
    )
    if key not in _NC_CACHE:
        _NC_CACHE[key] = build_nc(*key)
    return _NC_CACHE[key]


def run_on_hw(in_maps, T=T_FULL, trace=False, tmpdir=None):
    from concourse.bass_utils import run_bass_kernel_spmd

    nc = get_nc(T)
    return run_bass_kernel_spmd(
        nc, in_maps, list(range(NCORES)), trace=trace, tmpdir=tmpdir
    )


def kernel(x, Wq, bq, Wk, bk, Wv, bv):
    in_maps = host_prepare(x, Wq, bq, Wk, bk, Wv, bv)
    res = run_on_hw(in_maps)
    return host_combine(res.results)



# revision 12
# speedup vs baseline: 1.0098x; 1.0098x over previous
"""Trainium2 Bass kernel: single-head causal attention with softmax over the
QUERY axis (dim=1), as in the reference nn.Module.

Math per batch b:
    q = x Wq + bq ; k = x Wk + bk ; v = x Wv + bv          [T, H]
    S[i, j] = (q_i . k_j) * H**-0.5, masked to i >= j (tril)
    softmax over axis i (per key-column j):  s_j = sum_{i>=j} exp(S[i,j])
    out[i] = sum_j (exp(S[i,j]) / s_j) * v[j]
(no max-subtraction needed: |S| <= ~4 for this input distribution, exp is
exactly shift-invariant mathematically and fp32-safe here)

Sharding (8 cores): (batch b in 0..3) x (column-parity p in 0..1).
Core (b, p) owns key-columns j in 128-blocks {2t+p}. Column stats (s_j) are
core-local; partial outputs out^T[h, i] are summed on the host.

SPMD trick: all 8 cores run ONE program. Parity enters only through data:
 - for p=1 the host swaps adjacent 128-column blocks of x^T, so the core's
   kv-blocks always sit at even block positions (2s), and
 - a per-core [128, 256] additive mask (-40 on invalid) folded into the score
   PSUM via an identity matmul handles the triangular diagonal block plus the
   (invalid for p=1) neighbour block on each strip's first chunk.
The host un-swaps the p=1 output columns before combining.

v2 structure (vs v1): the tail after the last projection chunk is fully
pipelined: each strip's final exp unit feeds a per-strip finalize (sum,
reciprocal, v-scale) which unblocks that strip's output matmuls, emitted
strip-major into 4 concurrently-open PSUM accumulators (one per 1024 of i).
This keeps PE busy through the tail (HAM stays at 8/8) and overlaps the
ACT-bound exp tail with the output matmuls.
"""

import os
import sys

import numpy as np

for _p in ("/opt/trn_rl_repo", os.path.expanduser("~/.axon_site/_ro/trn_rl_repo")):
    if os.path.isdir(_p) and _p not in sys.path:
        sys.path.append(_p)

import concourse.bass as bass  # noqa: E402
import concourse.mybir as mybir  # noqa: E402
import concourse.tile as tile  # noqa: E402
from concourse import bacc  # noqa: E402

B, T_FULL, E, H = 4, 4096, 1024, 64
NCORES = 8

F32 = mybir.dt.float32
F32R = mybir.dt.float32r
BF16 = mybir.dt.bfloat16
AF = mybir.ActivationFunctionType
AX = mybir.AxisListType


def build_nc(T=T_FULL, xt_dt=BF16, seq_dt=BF16, tail_mode="pipe"):
    """Emit the single-core SPMD program. T must be a multiple of 1024."""
    assert T % 1024 == 0
    NEC = E // 128   # contraction chunks for the projections
    NCH = T // 512   # 512-wide i-chunks
    NST = T // 256   # kv strips owned by this core (one 128-block each)
    NIP = T // 1024  # 1024-wide DMA groups (2 i-chunks each)

    nc = bacc.Bacc(
        "TRN2", target_bir_lowering=False, debug=False, num_devices=NCORES
    )
    xt = nc.dram_tensor("xt", [E, T], xt_dt, kind="ExternalInput").ap()
    # [Wq*scale | Wk | Wv] prearranged on host into SBUF layout [p, ec, col]
    wgt = nc.dram_tensor(
        "wgt", [128, NEC * 192], xt_dt, kind="ExternalInput"
    ).ap()
    # packed seq-dtype consts:
    #   maskneg [128, 0:512] (0 valid / -40 invalid), ident128 [128, 512:640],
    #   id64 [0:64, 640:704]
    cseq = nc.dram_tensor("cseq", [128, 704], seq_dt, kind="ExternalInput").ap()
    # packed f32 consts: bqk [128, 0:1], bv [0:64, 1:2]
    cf32 = nc.dram_tensor("cf32", [128, 2], F32, kind="ExternalInput").ap()
    outp = nc.dram_tensor("outp", [H, T], F32, kind="ExternalOutput").ap()
    scr = nc.dram_tensor("scr", [128, 128], F32).ap()  # warmup sink (Internal)
    dbg = None
    if DEBUG_DUMP:
        dbg = nc.dram_tensor(
            "dbg", [128, 256 + 8 * (T // 256) + 2 * (T // 256)], F32,
            kind="ExternalOutput",
        ).ap()

    def strip_len(s):
        return T - 256 * s

    def head_w(s):
        return 512 if s % 2 == 0 else 256

    def units_of(s):
        """Scoring units: head (masked), then 1024-wide exp superchunks."""
        L = strip_len(s)
        offs = [0]
        o = head_w(s)
        while o < L:
            offs.append(o)
            o += 1024
        out = []
        for i, off in enumerate(offs):
            w = min(1024 if i else head_w(s), L - off)
            ready = (256 * s + off + w - 1) // 512
            out.append((i, off, w, ready))
        return out

    with tile.TileContext(nc) as tc:
        with (
            tc.tile_pool(name="consts", bufs=1) as consts,
            tc.tile_pool(name="big", bufs=1) as big,
            tc.tile_pool(name="xtp", bufs=3) as xtp,
            tc.tile_pool(name="outb", bufs=3) as outb,
        ):
            # --- constants ---
            wgt_sb = consts.tile([128, NEC, 192], xt_dt)
            wgt_r = wgt.rearrange("p (n c) -> p n c", n=NEC)
            # split: ec0 first so the first projection can start early
            nc.sync.dma_start(out=wgt_sb[:, 0:1, :], in_=wgt_r[:, 0:1, :])
            nc.sync.dma_start(out=wgt_sb[:, 1:NEC, :], in_=wgt_r[:, 1:NEC, :])
            wqk_sb = wgt_sb[:, :, 0:128]
            wv_sb = wgt_sb[:, :, 128:192]
            cf32_sb = consts.tile([128, 2], F32)
            nc.scalar.dma_start(out=cf32_sb, in_=cf32)
            bqk_sb = cf32_sb[:, 0:1]
            bv_sb = cf32_sb[0:64, 1:2]
            cseq_sb = consts.tile([128, 704], seq_dt)
            nc.scalar.dma_start(out=cseq_sb, in_=cseq)
            maskneg_sb = cseq_sb[:, 0:512]
            ident_sb = cseq_sb[:, 512:640]
            id_sb = cseq_sb[0:64, 640:704]

            # warmup stationary: memset, so no dependency on any DMA
            wtile = consts.tile([128, 128], seq_dt)
            nc.gpsimd.memset(wtile, 1.0)

            # --- persistent intermediates ---
            qkT = big.tile([128, T], seq_dt)      # rows 0:64 q'^T, 64:128 k^T
            kT_sb = big.tile([H, NST, 128], seq_dt)  # k^T kv blocks, base-0
            vT_kv = big.tile([H, NST, 128], seq_dt)  # v^T, kv blocks only
            v_sb = big.tile([128, NST, H], seq_dt)  # v[j, h], later scaled 1/s
            e_t = [
                big.tile([128, strip_len(s)], seq_dt, tag=f"e{s}", name=f"e{s}")
                for s in range(NST)
            ]
            sums = big.tile([128, NST, 8], F32)   # per-unit exp-sum partials
            stot = big.tile([128, NST, 1], F32)
            rec = big.tile([128, NST, 1], F32)
            nc.gpsimd.memset(sums, 0.0)

            wsink = big.tile([128, 128], F32)

            # classify units: per strip, the final unit (ready == NCH-1) is
            # emitted in the pipelined tail; the rest ride the ic loop.
            units_by_ic = {}
            tail_unit = {}
            for s in range(NST):
                for (i, off, w, ready) in units_of(s):
                    if ready == NCH - 1:
                        assert s not in tail_unit
                        tail_unit[s] = (i, off, w)
                    else:
                        units_by_ic.setdefault(ready, []).append((s, i, off, w))
            assert len(tail_unit) == NST

            with tc.tile_pool(name="psc", bufs=2, space="PSUM") as pscp:

                def emit_unit(s, idx, off, w, sum_dve):
                    """Scores+exp for cols [off, off+w) of strip s."""
                    sc = pscp.tile([128, 1024], F32, tag="sc", name=f"sc{s}_{idx}")
                    nseg = (w + 511) // 512
                    for si in range(nseg):
                        seg = 512 * si
                        sw = min(512, w - seg)
                        g = 256 * s + off + seg
                        # per-seg start resets this region's has_written bits
                        # (slots rotate; a unit-wide group would accumulate
                        # onto the previous occupant's stale scores)
                        nc.tensor.matmul(
                            sc[:, seg : seg + sw],
                            lhsT=kT_sb[:, s, :],
                            rhs=qkT[0:64, g : g + sw],
                            start=True,
                            stop=True,
                            skip_group_check=(idx == 0),
                        )
                    if idx == 0:
                        # fold the causal/neighbour mask into PSUM: only the
                        # first 256 cols can be invalid (additive -40)
                        mw = min(256, w)
                        nc.tensor.matmul(
                            sc[:, 0:mw],
                            lhsT=ident_sb,
                            rhs=maskneg_sb[:, 0:mw],
                            start=False,
                            stop=True,
                            skip_group_check=True,
                        )
                    if sum_dve:
                        nc.scalar.activation(
                            out=e_t[s][:, off : off + w],
                            in_=sc[:, 0:w],
                            func=AF.Exp,
                        )
                        nc.vector.reduce_sum(
                            out=sums[:, s, idx : idx + 1],
                            in_=e_t[s][:, off : off + w],
                            axis=AX.X,
                        )
                    else:
                        nc.scalar.activation(
                            out=e_t[s][:, off : off + w],
                            in_=sc[:, 0:w],
                            func=AF.Exp,
                            accum_out=sums[:, s, idx : idx + 1],
                        )

                n_mid = 0

                def emit_units_of_ic(ic):
                    nonlocal n_mid
                    for (s_, i, off, w) in units_by_ic.get(ic, []):
                        if i == 0:
                            emit_unit(s_, i, off, w, sum_dve=True)
                        else:
                            # alternate mid-unit sums between DVE and ACT to
                            # balance the two engines
                            emit_unit(s_, i, off, w, sum_dve=(n_mid % 2 == 0))
                            n_mid += 1

                with (
                    tc.tile_pool(name="pqk", bufs=2, space="PSUM") as pqk,
                    tc.tile_pool(name="pvt", bufs=1, space="PSUM") as pvt,
                    tc.tile_pool(name="ptr", bufs=1, space="PSUM") as ptr,
                ):

                    def emit_warmup(n):
                        """Dummy back-to-back matmuls to trip the PE HAM clock
                        gate to 8/8 while PE would otherwise idle."""
                        wp = pscp.tile([128, 1024], F32, tag="sc", name="warm")
                        for i in range(n):
                            nc.tensor.matmul(
                                wp[:, 0:128],
                                lhsT=wtile,
                                rhs=wtile,
                                start=(i == 0),
                                stop=(i == n - 1),
                            )
                        nc.vector.tensor_copy(out=wsink, in_=wp[:, 0:128])

                    emit_warmup(28)

                    # --- phase 1: projections + scoring, pipelined by ic ---
                    for icp in range(NIP):
                        xts = xtp.tile(
                            [128, NEC, 1024], xt_dt, tag="xt", name=f"xts{icp}"
                        )
                        xt_r = xt.rearrange("(n p) t -> p n t", p=128)[
                            :, :, 1024 * icp : 1024 * (icp + 1)
                        ]
                        if icp == 0:
                            # split first load so matmuls start sooner
                            nc.sync.dma_start(
                                out=xts[:, 0:1, :], in_=xt_r[:, 0:1, :]
                            )
                            nc.sync.dma_start(
                                out=xts[:, 1:NEC, :], in_=xt_r[:, 1:NEC, :]
                            )
                        else:
                            nc.sync.dma_start(out=xts, in_=xt_r)
                        for sub in range(2):
                            ic = 2 * icp + sub
                            qk_ps = pqk.tile([128, 512], F32, tag="qk")
                            vt_ps = pvt.tile([H, 256], F32, tag="vt")
                            for ec in range(NEC):
                                rhs = xts[:, ec, 512 * sub : 512 * sub + 512]
                                nc.tensor.matmul(
                                    qk_ps,
                                    lhsT=wqk_sb[:, ec, :],
                                    rhs=rhs,
                                    start=(ec == 0),
                                    stop=(ec == NEC - 1),
                                )
                                kv_rhs = rhs.rearrange("p (b x) -> p b x", b=2)[
                                    :, :, 0:128
                                ]
                                nc.tensor.matmul(
                                    vt_ps,
                                    lhsT=wv_sb[:, ec, :],
                                    rhs=kv_rhs,
                                    start=(ec == 0),
                                    stop=(ec == NEC - 1),
                                )
                            nc.vector.tensor_scalar_add(
                                out=qkT[:, 512 * ic : 512 * (ic + 1)],
                                in0=qk_ps,
                                scalar1=bqk_sb,
                            )
                            nc.vector.tensor_scalar_add(
                                out=vT_kv[:, 2 * ic : 2 * ic + 2, :],
                                in0=vt_ps,
                                scalar1=bv_sb,
                            )
                            # k^T blocks of this chunk to base-0 (one DMA, 2 strips)
                            nc.sync.dma_start(
                                out=kT_sb[:, 2 * ic : 2 * ic + 2, :],
                                in_=qkT[
                                    64:128, 512 * ic : 512 * (ic + 1)
                                ].rearrange("p (b x) -> p b x", b=2)[:, :, 0:128],
                            )
                            # v blocks of this chunk -> [j, h] via PE transpose
                            tr = ptr.tile([128, 2, H], seq_dt, tag="tr")
                            for m in range(2):
                                nc.tensor.transpose(
                                    tr[:, m, :], vT_kv[:, 2 * ic + m, :], id_sb
                                )
                            nc.vector.tensor_copy(
                                out=v_sb[:, 2 * ic : 2 * ic + 2, :], in_=tr
                            )
                            emit_units_of_ic(ic)

                # --- pipelined tail: per-strip last exp unit -> finalize ->
                # strip-major output matmuls into 4 concurrent accumulators ---
                with tc.tile_pool(name="pout", bufs=1, space="PSUM") as pout:
                    op = [
                        pout.tile([128, 512], F32, tag=f"op{k}", name=f"op{k}")
                        for k in range(NCH // 2)
                    ]

                    def emit_tail(s):
                        idx, off, w = tail_unit[s]
                        emit_unit(s, idx, off, w, sum_dve=True)
                        # per-strip finalize: total, reciprocal, scale v
                        nc.vector.reduce_sum(
                            out=stot[:, s, :], in_=sums[:, s, :], axis=AX.X
                        )
                        nc.vector.reciprocal(out=rec[:, s, :], in_=stot[:, s, :])
                        nc.vector.tensor_scalar_mul(
                            out=v_sb[:, s, :],
                            in0=v_sb[:, s, :],
                            scalar1=rec[:, s, :],
                        )

                    def emit_out_group(s):
                        """All output matmuls of strip s (chunks c >= s//2)."""
                        for c in range(s // 2, NCH):
                            k, half = divmod(c, 2)
                            rows = op[k][64 * half : 64 * half + 64, :]
                            soff = 512 * c - 256 * s
                            if soff >= 0:
                                dst, rhs = rows, e_t[s][:, soff : soff + 512]
                            else:
                                dst, rhs = rows[:, 256:512], e_t[s][:, 0:256]
                            nc.tensor.matmul(
                                dst,
                                lhsT=v_sb[:, s, :],
                                rhs=rhs,
                                start=(s == 0),
                                stop=(s == 2 * c + 1),
                                skip_group_check=True,
                            )

                    def emit_pair_out(k):
                        """Copy finished PSUM pair k and DMA both chunks out."""
                        ob = outb.tile([128, 512], F32, tag="ob")
                        nc.scalar.activation(out=ob, in_=op[k], func=AF.Copy)
                        for m in range(2):
                            cidx = 2 * k + m
                            nc.sync.dma_start(
                                out=outp[:, 512 * cidx : 512 * (cidx + 1)],
                                in_=ob[64 * m : 64 * m + 64, :],
                            )

                    if tail_mode == "pipe":
                        PRE = 5
                        for s in range(min(PRE, NST)):
                            emit_tail(s)
                        for s in range(NST):
                            emit_out_group(s)
                            if s + PRE < NST:
                                emit_tail(s + PRE)
                            if s % 4 == 3:
                                emit_pair_out(s // 4)
                    else:
                        # v1-style: all tails, barrier finalize, pair-major
                        for s in range(NST):
                            idx, off, w = tail_unit[s]
                            emit_unit(s, idx, off, w, sum_dve=True)
                        nc.vector.reduce_sum(out=stot, in_=sums, axis=AX.X)
                        nc.vector.reciprocal(out=rec, in_=stot)
                        for s in range(NST):
                            nc.vector.tensor_scalar_mul(
                                out=v_sb[:, s, :],
                                in0=v_sb[:, s, :],
                                scalar1=rec[:, s, :],
                            )
                        for k in range(NCH // 2):
                            c0, c1 = 2 * k, 2 * k + 1
                            mms = []
                            for s in range(2 * c1 + 2):
                                for half, c in ((0, c0), (1, c1)):
                                    if s > 2 * c + 1:
                                        continue
                                    soff = 512 * c - 256 * s
                                    rows = op[k][64 * half : 64 * half + 64, :]
                                    if soff >= 0:
                                        mms.append(
                                            (half, rows, s,
                                             e_t[s][:, soff : soff + 512])
                                        )
                                    else:
                                        mms.append(
                                            (half, rows[:, 256:512], s,
                                             e_t[s][:, 0:256])
                                        )
                            seen_half = set()
                            for i, (half, dst, s, rhs) in enumerate(mms):
                                first = half not in seen_half
                                seen_half.add(half)
                                nc.tensor.matmul(
                                    dst,
                                    lhsT=v_sb[:, s, :],
                                    rhs=rhs,
                                    start=first,
                                    stop=(i == len(mms) - 1),
                                    skip_group_check=True,
                                )
                            emit_pair_out(k)

            if dbg is not None:
                e15f = big.tile([128, 256], F32)
                nc.vector.tensor_copy(out=e15f, in_=e_t[15])
                nc.sync.dma_start(out=dbg[:, 0:256], in_=e15f)
                nc.sync.dma_start(
                    out=dbg[:, 256 : 256 + 8 * NST],
                    in_=sums.rearrange("p a b -> p (a b)"),
                )
                nc.sync.dma_start(
                    out=dbg[:, 256 + 8 * NST : 256 + 9 * NST],
                    in_=stot.rearrange("p a b -> p (a b)"),
                )
                nc.sync.dma_start(
                    out=dbg[:, 256 + 9 * NST : 256 + 10 * NST],
                    in_=rec.rearrange("p a b -> p (a b)"),
                )
            nc.sync.dma_start(out=scr, in_=wsink)

    nc.compile()
    return nc


def _make_mask(parity):
    m = np.zeros((128, 512), np.float32)
    m[:, 0:128] = np.tri(128, dtype=np.float32).T  # valid: i_off >= j_off
    if parity == 0:
        m[:, 128:256] = 1.0
    m[:, 256:512] = 1.0
    return m


def _swap_blocks_cols(a, blk=128):
    """Swap adjacent blk-wide column blocks: [..., 2t | 2t+1] -> [2t+1 | 2t]."""
    n = a.shape[-1]
    v = a.reshape(*a.shape[:-1], n // (2 * blk), 2, blk)
    return np.ascontiguousarray(v[..., ::-1, :].reshape(a.shape))


def host_prepare(x, Wq, bq, Wk, bk, Wv, bv, T=T_FULL, xt_dt=None, seq_dt=None):
    xt_np = mybir.dt.np(xt_dt if xt_dt is not None else XT_DT)
    seq_np = mybir.dt.np(seq_dt if seq_dt is not None else SEQ_DT)
    scale = np.float32(H**-0.5)
    # [Wq*scale | Wk | Wv] -> SBUF layout [p, ec*192 + col]
    wcat = np.concatenate(
        [
            np.asarray(Wq, np.float32) * scale,
            np.asarray(Wk, np.float32),
            np.asarray(Wv, np.float32),
        ],
        axis=1,
    )  # [E, 192]
    wgt_h = np.ascontiguousarray(
        wcat.reshape(8, 128, 192).transpose(1, 0, 2).reshape(128, 8 * 192)
    ).astype(xt_np)
    cf32_h = np.zeros((128, 2), np.float32)
    cf32_h[:, 0] = np.concatenate(
        [np.asarray(bq, np.float32) * scale, np.asarray(bk, np.float32)]
    )
    cf32_h[0:H, 1] = np.asarray(bv, np.float32)
    cseq = np.zeros((128, 704), np.float32)
    cseq[:, 512:640] = np.eye(128, dtype=np.float32)
    cseq[0:H, 640:704] = np.eye(H, dtype=np.float32)
    cseq_m = [cseq.copy(), cseq.copy()]
    for p in (0, 1):
        cseq_m[p][:, 0:512] = (1.0 - _make_mask(p)) * (-40.0)
    in_maps = []
    for core in range(NCORES):
        b, p = divmod(core, 2)
        xt_h = np.ascontiguousarray(np.asarray(x[b], np.float32).T)  # [E, T]
        if p == 1:
            xt_h = _swap_blocks_cols(xt_h)
        in_maps.append(
            {
                "xt": xt_h.astype(xt_np),
                "wgt": wgt_h,
                "cseq": cseq_m[p].astype(seq_np),
                "cf32": cf32_h,
            }
        )
    return in_maps


def host_combine(results, T=T_FULL):
    out = np.zeros((B, T, H), np.float32)
    for b in range(B):
        o0 = np.asarray(results[2 * b]["outp"])
        o1 = _swap_blocks_cols(np.asarray(results[2 * b + 1]["outp"]))
        out[b] = (o0 + o1).T
    return out


_NC_CACHE = {}

# active dtype mode for matmul operands (PSUM accumulation stays fp32)
XT_DT = BF16
SEQ_DT = BF16


TAIL_MODE = "pipe"
DEBUG_DUMP = False


def get_nc(T=T_FULL, xt_dt=None, seq_dt=None):
    key = (
        T,
        xt_dt if xt_dt is not None else XT_DT,
        seq_dt if seq_dt is not None else SEQ_DT,
        TAIL_MODE,
        DEBUG_DUMP,
    )
    if key not in _NC_CACHE:
        _NC_CACHE[key] = build_nc(*key[:3], tail_mode=TAIL_MODE)
    return _NC_CACHE[key]


def run_on_hw(in_maps, T=T_FULL, trace=False, tmpdir=None):
    from concourse.bass_utils import run_bass_kernel_spmd

    nc = get_nc(T)
    return run_bass_kernel_spmd(
        nc, in_maps, list(range(NCORES)), trace=trace, tmpdir=tmpdir
    )


def kernel(x, Wq, bq, Wk, bk, Wv, bv):
    in_maps = host_prepare(x, Wq, bq, Wk, bk, Wv, bv)
    res = run_on_hw(in_maps)
    return host_combine(res.results)
